# revision 25
# baseline (speedup 1.0000x reference)
"""GAT layer (N=8192, D=64) as a Bass/Tile kernel on 8 TRN2 NeuronCores.

Math (reference):
    h  = x @ W.T + b
    s1 = h @ a1 ; s2 = h @ a2                    # [N] each
    score[i,j] = s2[i] + s1[j]
    att = softmax_j(leaky_relu(score))
    out = att @ x

Reformulation used here:
    Fold the linear layer:  v = W.T @ [a1|a2], c_k = b.a_k
      p1 = x @ v1 ; p2 = x @ v2 ; s1 = p1 + c1 ; s2 = p2 + c2
    Softmax rows are shift invariant, so subtract p2[i] from row i:
      exp(lr(score) - p2[i]) = max( exp(sh1[j]),
                                    exp(0.01*sh1[j]) * exp(-0.99*p2[i]) )
      with sh1[j] = p1[j] + c1 + c2   (lr = leaky-relu, exp is monotone
      so exp(max(a,b)) = max(exp a, exp b))
    So with per-j-row scalars E1 = exp(sh1), F1 = exp(0.01*sh1) and a
    broadcast tile G2b[j,i] = exp(-0.99*p2[i]), the unnormalized weight
    tile (layout [j partitions, i free]) is ONE tensor_scalar op:
      e[j,i] = max( G2b[j,i] * F1[j],  E1[j] )
    The final matmul (with a ones-column appended to x to get the
    softmax denominator for free) accumulates over j in PSUM:
      outT[0:64, i] += x_ext[j,:].T @ e[j, i] ; Z[i] = outT[64, i]

Sharding: each core owns N/8 = 1024 query rows i (full x is only 2MB and
is replicated to every core), no collectives needed.

Engine plan (v3):
  - DVE: the 64 fused mult+max e-tiles (critical stream) + 8 per-chunk
    s1 reduces + small prologue pieces.
  - PE: 128 accumulating matmuls, v/vb/G2b projections, epilogue
    transposes.
  - GPSIMD: the 8 s1 product chunks (x * v1 broadcast), identity build.
  - ACT: exps, fp32->bf16 x casts (replaces a 2MB xbf DMA), PSUM->SBUF
    copies, half the epilogue normalizes.
  Prologue latency is minimized by computing v1-broadcast and c12 with a
  single ones-matmul over DVE-premultiplied columns (no transpose chain).
"""

import sys
import types

import ml_dtypes
import numpy as np

import concourse.bacc as bacc
import concourse.bass as bass
import concourse.mybir as mybir
import concourse.tile as tile
from concourse.bass_utils import run_bass_kernel_spmd
from concourse.masks import make_identity


def _install_ntff_hook_shim():
    """The agent image's ``antenv`` lacks ``axon_hooks``; provide it so
    ``run_bass_kernel_spmd(trace=True)`` can capture NTFF profiles. The
    actual hook implementation ships with the axon boot package."""
    if "antenv.axon_hooks" in sys.modules:
        return
    try:
        from trn_agent_boot.trn_boot import _ntff_profile_via_ctypes

        hook = _ntff_profile_via_ctypes("/opt/axon/libaxon_pjrt.so")
        mod = types.ModuleType("antenv.axon_hooks")
        mod._hook = hook
        mod.get_axon_ntff_profile_hook = lambda: mod._hook
        mod.set_axon_ntff_profile_hook = lambda h: setattr(mod, "_hook", h)
        sys.modules["antenv.axon_hooks"] = mod
    except Exception:
        pass


_install_ntff_hook_shim()

N, D = 8192, 64
NCORES = 8
RB = N // NCORES          # rows (i) per core = 1024
NT = N // 128             # j tiles of 128 = 64
BT = RB // 128            # i tiles per core = 8
F32 = mybir.dt.float32
BF16 = mybir.dt.bfloat16
EXP = mybir.ActivationFunctionType.Exp
ADD = mybir.AluOpType.add
MUL = mybir.AluOpType.mult
MAX = mybir.AluOpType.max
AX_X = mybir.AxisListType.X
PKW = D + 3  # packed small-input width (W | b | a1 | a2)
CH = 8       # j tiles per chunk


def build_bass() -> bass.Bass:
    nc = bacc.Bacc(None)
    # partition-major (p, t, d) layout for x, prepared on the host
    xp_d = nc.declare_dram_parameter("xp", [128, NT * D], F32, isOutput=False)
    pk_d = nc.declare_dram_parameter("pack", [D, PKW], F32, isOutput=False)
    xbkT_d = nc.declare_dram_parameter("xbkT", [D, 2 * RB], BF16, isOutput=False)
    out_d = nc.declare_dram_parameter("out", [128, BT * D], F32, isOutput=True)

    with tile.TileContext(nc) as tc:
        with (
            tc.tile_pool(name="persist", bufs=1) as persist,
            tc.tile_pool(name="small", bufs=1) as small,
            tc.tile_pool(name="work", bufs=5) as work,
            tc.tile_pool(name="epool", bufs=14) as epool,
            tc.tile_pool(name="opool", bufs=4) as opool,
            tc.tile_pool(name="psumA", bufs=3, space="PSUM") as psumA,
            tc.tile_pool(name="psumB", bufs=1, space="PSUM") as psumB,
        ):
            # ---------------- input DMAs, priority order ----------------
            pk = small.tile([D, PKW], F32)
            nc.sync.dma_start(pk, pk_d[:, :])
            # bf16 hi|lo split of x-block.T (cols 0:RB hi, RB:2RB lo)
            xbkT_sb = small.tile([D, 2 * RB], BF16)
            nc.sync.dma_start(xbkT_sb, xbkT_d[:, :])
            x_flat = persist.tile([128, NT * D], F32)
            x_sb = x_flat.rearrange("p (t d) -> p t d", t=NT)
            xchunks = [(0, 4), (4, 4), (8, 8), (16, 16), (32, 16), (48, 16)]
            for tw, nw in xchunks:
                nc.sync.dma_start(
                    x_flat[:, tw * D : (tw + nw) * D],
                    xp_d[:, tw * D : (tw + nw) * D],
                )

            W_sb = pk[0:D, 0:D]
            b_col = pk[0:D, D : D + 1]
            a_sb = pk[0:D, D + 1 : D + 3]
            a1col = pk[0:D, D + 1 : D + 2]
            a2col = pk[0:D, D + 2 : D + 3]

            # ---------------- on-device constants ----------------
            ones64 = small.tile([D, 128], F32)
            nc.vector.memset(ones64, 1.0)
            ones_bf = small.tile([1, 128], BF16)
            nc.vector.memset(ones_bf, 1.0)
            ident = small.tile([128, 128], F32)
            make_identity(nc, ident)  # gpsimd memset + affine_select

            # bf16 x_ext (x | ones | garbage), derived on device. Columns
            # 65..127 stay uninitialized: they only feed PSUM partitions
            # 65..127 which are never read.
            xbf_flat = persist.tile([128, NT * 128], BF16)
            x_bf = xbf_flat.rearrange("p (t d) -> p t d", t=NT)
            nc.vector.memset(x_bf[:, :, D : D + 1], 1.0)  # ones column
            # ACT warmup: trigger the activation table load at engine boot
            warm = small.tile([1, 1], F32)
            nc.vector.memset(warm, 0.0)
            warm2 = small.tile([1, 1], F32)
            nc.scalar.activation(out=warm2, in_=warm, func=EXP)

            # ---------------- v1 broadcast + c12, one ones-matmul ---------
            # vb_rhs = [ a1*W | b*(a1+a2) ]  (per-row scaling on DVE)
            # vb_ps[p, 0:64] = sum_k a1[k] W[k, d] = v1[d]  (all partitions)
            # vb_ps[p, 64]   = sum_k b[k](a1[k]+a2[k]) = c1+c2
            vb_rhs = small.tile([D, D + 1], F32)
            nc.vector.tensor_scalar(
                out=vb_rhs[:, 0:D], in0=W_sb, scalar1=a1col,
                scalar2=None, op0=MUL,
            )
            a12 = small.tile([D, 1], F32)
            nc.vector.tensor_add(a12, a1col, a2col)
            nc.vector.tensor_mul(vb_rhs[:, D : D + 1], b_col, a12)
            vb_ps = psumA.tile([128, D + 1], F32, tag="ps", name="vb_ps")
            nc.tensor.matmul(vb_ps, lhsT=ones64, rhs=vb_rhs, start=True, stop=True)
            vb_sb = small.tile([128, D + 1], F32)
            nc.scalar.copy(out=vb_sb, in_=vb_ps)
            v1b = vb_sb[:, 0:D]
            c12 = vb_sb[:, D : D + 1]
            c12s = small.tile([128, 1], F32)
            nc.vector.tensor_scalar(
                out=c12s, in0=c12, scalar1=0.01, scalar2=None, op0=MUL
            )


            # v2 column for the p2 projection, split bf16 hi/lo
            v_ps = psumA.tile([D, 2], F32, tag="ps", name="v_ps")
            nc.tensor.matmul(v_ps, lhsT=W_sb, rhs=a_sb, start=True, stop=True)
            v_sb = small.tile([D, 2], F32)
            nc.scalar.copy(out=v_sb, in_=v_ps)
            v2h = small.tile([D, 1], BF16)
            nc.vector.tensor_copy(v2h, v_sb[:, 1:2])
            v2hf = small.tile([D, 1], F32)
            nc.vector.tensor_copy(v2hf, v2h)
            v2lf = small.tile([D, 1], F32)
            nc.vector.tensor_sub(v2lf, v_sb[:, 1:2], v2hf)
            v2l = small.tile([D, 1], BF16)
            nc.vector.tensor_copy(v2l, v2lf)

            # ---------------- p2 for this block -> G2b ----------------
            # p2row = v2.T @ xblk.T ; G2b[j,i] = exp(-0.99 * p2[i]) bcast
            G2b = persist.tile([128, RB], BF16)
            QW = 256
            for q in range(4):
                p2r_ps = psumA.tile([1, QW], F32, tag="ps", name="p2r_ps")
                # p2 = v2h.xh + v2l.xh + v2h.xl  (v2l.xl term negligible)
                nc.tensor.matmul(
                    p2r_ps,
                    lhsT=v2h,
                    rhs=xbkT_sb[:, q * QW : (q + 1) * QW],
                    start=True,
                    stop=False,
                )
                nc.tensor.matmul(
                    p2r_ps,
                    lhsT=v2l,
                    rhs=xbkT_sb[:, q * QW : (q + 1) * QW],
                    start=False,
                    stop=False,
                )
                nc.tensor.matmul(
                    p2r_ps,
                    lhsT=v2h,
                    rhs=xbkT_sb[:, RB + q * QW : RB + (q + 1) * QW],
                    start=False,
                    stop=True,
                )
                g2row = small.tile([1, QW], BF16, tag="g2row", name="g2row")
                nc.scalar.activation(out=g2row, in_=p2r_ps, func=EXP, scale=-0.99)
                gb_ps = psumA.tile([128, QW], F32, tag="ps", name="gb_ps")
                nc.tensor.matmul(
                    gb_ps, lhsT=ones_bf, rhs=g2row, start=True, stop=True
                )
                nc.vector.tensor_copy(G2b[:, q * QW : (q + 1) * QW], gb_ps)

            # ---------------- s1 products on GPSIMD (runs ahead) ----------
            # tmp[p, t, d] = x[p, t, d] * v1[d]
            schunks = [(0, 4), (4, 4)] + [(8 * c, 8) for c in range(1, 8)]

            def s1_products(ci):
                tw, nw = schunks[ci]
                v1b_b = bass.AP(
                    tensor=vb_sb.tensor,
                    offset=vb_sb.offset,
                    ap=[vb_sb.ap[0], [0, nw], [1, D]],
                )
                tmp = work.tile([128, nw, D], F32, tag="tmp", name="tmp")
                nc.gpsimd.tensor_mul(tmp, x_sb[:, tw : tw + nw, :], v1b_b)
                return tmp

            tmps = [s1_products(0), s1_products(1), s1_products(2), s1_products(3)]

            # ---------------- ACT: x casts, chunk order -------------------
            s1c = small.tile([128, NT], F32)
            E1c = small.tile([128, NT], F32)
            F1c = small.tile([128, NT], F32)
            for tw, nw in schunks:
                # cast x chunk -> bf16 x_ext columns 0:64
                nc.scalar.copy(
                    out=x_bf[:, tw : tw + nw, 0:D],
                    in_=x_sb[:, tw : tw + nw, :],
                )

            # -------- main stream --------------------------------------
            # s1 reduction runs on ACT (Identity + accum_out, one per
            # j-tile) so the DVE queue holds ONLY the 64 e-tiles.
            SF = D + 1
            acc0 = psumB.tile([SF, 512], F32, tag="acc0", name="acc0")
            acc1 = psumB.tile([SF, 512], F32, tag="acc1", name="acc1")
            accs = [acc0, acc1]
            for ci, (tw, nw) in enumerate(schunks):
                nc.vector.tensor_reduce(
                    out=s1c[:, tw : tw + nw],
                    in_=tmps[ci],
                    axis=AX_X,
                    op=ADD,
                )
                nc.scalar.activation(
                    out=E1c[:, tw : tw + nw],
                    in_=s1c[:, tw : tw + nw],
                    func=EXP,
                    bias=c12,
                    scale=1.0,
                )
                nc.scalar.activation(
                    out=F1c[:, tw : tw + nw],
                    in_=s1c[:, tw : tw + nw],
                    func=EXP,
                    bias=c12s,
                    scale=0.01,
                )
                if ci + 4 < len(schunks):
                    tmps.append(s1_products(ci + 4))
                for jt in range(tw, tw + nw):
                    if ci == 0:
                        # first chunk: split per G2b half so the stream can
                        # start before the last G2b quarter lands
                        for h in range(2):
                            e_h = epool.tile(
                                [128, 512], BF16, tag="eh", name="e_h"
                            )
                            nc.vector.tensor_scalar(
                                out=e_h,
                                in0=G2b[:, h * 512 : (h + 1) * 512],
                                scalar1=F1c[:, jt : jt + 1],
                                scalar2=E1c[:, jt : jt + 1],
                                op0=MUL,
                                op1=MAX,
                            )
                            nc.tensor.matmul(
                                accs[h],
                                lhsT=x_bf[:, jt, 0:SF],
                                rhs=e_h,
                                start=(jt == 0),
                                stop=(jt == NT - 1),
                            )
                    else:
                        e_t = epool.tile([128, RB], BF16, tag="e", name="e_t")
                        # e[j,i] = max(G2b[j,i] * F1[j], E1[j])
                        nc.vector.tensor_scalar(
                            out=e_t,
                            in0=G2b,
                            scalar1=F1c[:, jt : jt + 1],
                            scalar2=E1c[:, jt : jt + 1],
                            op0=MUL,
                            op1=MAX,
                        )
                        for h in range(2):
                            nc.tensor.matmul(
                                accs[h],
                                lhsT=x_bf[:, jt, 0:SF],
                                rhs=e_t[:, h * 512 : (h + 1) * 512],
                                start=(jt == 0),
                                stop=(jt == NT - 1),
                            )

            # ---------------- epilogue: normalize + store ----------------
            outT = small.tile([D + 1, RB], F32)
            for h in range(2):
                for qq in range(2):
                    nc.scalar.copy(
                        out=outT[
                            :, h * 512 + qq * 256 : h * 512 + (qq + 1) * 256
                        ],
                        in_=accs[h][:, qq * 256 : (qq + 1) * 256],
                    )
            out_flat = small.tile([128, BT * D], F32)
            out_sb = out_flat.rearrange("p (t d) -> p t d", t=BT)
            for t in range(BT):
                tp2 = psumA.tile([128, D + 1], F32, tag="ps", name="tp2")
                nc.tensor.transpose(
                    tp2, outT[:, t * 128 : (t + 1) * 128], ident[: D + 1, : D + 1]
                )
                rcol = opool.tile([128, 1], F32, tag="rcol", name="rcol")
                nc.vector.reciprocal(rcol, tp2[:, D : D + 1])
                if t % 2 == 0:
                    nc.vector.tensor_scalar(
                        out=out_sb[:, t, :],
                        in0=tp2[:, 0:D],
                        scalar1=rcol,
                        scalar2=None,
                        op0=MUL,
                    )
                else:
                    nc.scalar.mul(out_sb[:, t, :], tp2[:, 0:D], rcol)
                if t == 3:
                    nc.sync.dma_start(out_d[:, : 4 * D], out_flat[:, : 4 * D])
                elif t == 6:
                    nc.sync.dma_start(
                        out_d[:, 4 * D : 7 * D], out_flat[:, 4 * D : 7 * D]
                    )
            nc.sync.dma_start(out_d[:, 7 * D :], out_flat[:, 7 * D :])

    nc.finalize()
    return nc


def _execute(inputs: dict, trace: bool = False):
    x = np.ascontiguousarray(np.asarray(inputs["x"], dtype=np.float32))
    W = np.ascontiguousarray(np.asarray(inputs["W"], dtype=np.float32))
    b = np.ascontiguousarray(
        np.asarray(inputs["b"], dtype=np.float32).reshape(D, 1)
    )
    a = np.ascontiguousarray(
        np.asarray(inputs["a"], dtype=np.float32).reshape(2 * D, 1)
    )
    assert x.shape == (N, D) and W.shape == (D, D)

    # partition-major permutation: (t*128+p, d) -> (p, t*D+d)
    xp = np.ascontiguousarray(
        x.reshape(NT, 128, D).transpose(1, 0, 2).reshape(128, NT * D)
    )
    nc = build_bass()
    pack0 = np.zeros((D, PKW), np.float32)
    pack0[:, 0:D] = W
    pack0[:, D] = b[:, 0]
    pack0[:, D + 1] = a[:D, 0]
    pack0[:, D + 2] = a[D:, 0]
    in_maps = []
    for c in range(NCORES):
        xT = np.ascontiguousarray(x[c * RB : (c + 1) * RB].T)
        xh = xT.astype(ml_dtypes.bfloat16)
        xl = (xT - xh.astype(np.float32)).astype(ml_dtypes.bfloat16)
        xbkT = np.ascontiguousarray(np.concatenate([xh, xl], axis=1))
        in_maps.append({"xp": xp, "pack": pack0, "xbkT": xbkT})
    res = run_bass_kernel_spmd(
        nc, in_maps, core_ids=list(range(NCORES)), trace=trace
    )
    # un-permute each core's output: (p, t*D+d) -> (t*128+p, d)
    outs = []
    for r in res.results:
        o = r["out"].reshape(128, BT, D).transpose(1, 0, 2).reshape(RB, D)
        outs.append(o)
    out = np.ascontiguousarray(np.concatenate(outs, axis=0))
    return out, res


def kernel(x, W, b, a):
    out, _ = _execute({"x": x, "W": W, "b": b, "a": a})
    return out


# revision 26
# speedup vs baseline: 1.0138x; 1.0138x over previous
"""GAT layer (N=8192, D=64) as a Bass/Tile kernel on 8 TRN2 NeuronCores.

Math (reference):
    h  = x @ W.T + b
    s1 = h @ a1 ; s2 = h @ a2                    # [N] each
    score[i,j] = s2[i] + s1[j]
    att = softmax_j(leaky_relu(score))
    out = att @ x

Reformulation used here:
    Fold the linear layer:  v = W.T @ [a1|a2], c_k = b.a_k
      p1 = x @ v1 ; p2 = x @ v2 ; s1 = p1 + c1 ; s2 = p2 + c2
    Softmax rows are shift invariant, so subtract p2[i] from row i:
      exp(lr(score) - p2[i]) = max( exp(sh1[j]),
                                    exp(0.01*sh1[j]) * exp(-0.99*p2[i]) )
      with sh1[j] = p1[j] + c1 + c2   (lr = leaky-relu, exp is monotone
      so exp(max(a,b)) = max(exp a, exp b))
    So with per-j-row scalars E1 = exp(sh1), F1 = exp(0.01*sh1) and a
    broadcast tile G2b[j,i] = exp(-0.99*p2[i]), the unnormalized weight
    tile (layout [j partitions, i free]) is ONE tensor_scalar op:
      e[j,i] = max( G2b[j,i] * F1[j],  E1[j] )
    The final matmul (with a ones-column appended to x to get the
    softmax denominator for free) accumulates over j in PSUM:
      outT[0:64, i] += x_ext[j,:].T @ e[j, i] ; Z[i] = outT[64, i]

Sharding: each core owns N/8 = 1024 query rows i (full x is only 2MB and
is replicated to every core), no collectives needed.

Engine plan (v3):
  - DVE: the 64 fused mult+max e-tiles (critical stream) + 8 per-chunk
    s1 reduces + small prologue pieces.
  - PE: 128 accumulating matmuls, v/vb/G2b projections, epilogue
    transposes.
  - GPSIMD: the 8 s1 product chunks (x * v1 broadcast), identity build.
  - ACT: exps, fp32->bf16 x casts (replaces a 2MB xbf DMA), PSUM->SBUF
    copies, half the epilogue normalizes.
  Prologue latency is minimized by computing v1-broadcast and c12 with a
  single ones-matmul over DVE-premultiplied columns (no transpose chain).
"""

import sys
import types

import ml_dtypes
import numpy as np

import concourse.bacc as bacc
import concourse.bass as bass
import concourse.mybir as mybir
import concourse.tile as tile
from concourse.bass_utils import run_bass_kernel_spmd
from concourse.masks import make_identity


def _install_ntff_hook_shim():
    """The agent image's ``antenv`` lacks ``axon_hooks``; provide it so
    ``run_bass_kernel_spmd(trace=True)`` can capture NTFF profiles. The
    actual hook implementation ships with the axon boot package."""
    if "antenv.axon_hooks" in sys.modules:
        return
    try:
        from trn_agent_boot.trn_boot import _ntff_profile_via_ctypes

        hook = _ntff_profile_via_ctypes("/opt/axon/libaxon_pjrt.so")
        mod = types.ModuleType("antenv.axon_hooks")
        mod._hook = hook
        mod.get_axon_ntff_profile_hook = lambda: mod._hook
        mod.set_axon_ntff_profile_hook = lambda h: setattr(mod, "_hook", h)
        sys.modules["antenv.axon_hooks"] = mod
    except Exception:
        pass


_install_ntff_hook_shim()

N, D = 8192, 64
NCORES = 8
RB = N // NCORES          # rows (i) per core = 1024
NT = N // 128             # j tiles of 128 = 64
BT = RB // 128            # i tiles per core = 8
F32 = mybir.dt.float32
BF16 = mybir.dt.bfloat16
EXP = mybir.ActivationFunctionType.Exp
ADD = mybir.AluOpType.add
MUL = mybir.AluOpType.mult
MAX = mybir.AluOpType.max
AX_X = mybir.AxisListType.X
PKW = D + 3  # packed small-input width (W | b | a1 | a2)
CH = 8       # j tiles per chunk


def build_bass() -> bass.Bass:
    nc = bacc.Bacc(None)
    # partition-major (p, t, d) layout for x, prepared on the host
    xp_d = nc.declare_dram_parameter("xp", [128, NT * D], F32, isOutput=False)
    pk_d = nc.declare_dram_parameter("pack", [D, PKW], F32, isOutput=False)
    xbkT_d = nc.declare_dram_parameter("xbkT", [D, RB], F32, isOutput=False)
    out_d = nc.declare_dram_parameter("out", [128, BT * D], F32, isOutput=True)

    with tile.TileContext(nc) as tc:
        with (
            tc.tile_pool(name="persist", bufs=1) as persist,
            tc.tile_pool(name="small", bufs=1) as small,
            tc.tile_pool(name="work", bufs=3) as work,
            tc.tile_pool(name="epool", bufs=12) as epool,
            tc.tile_pool(name="opool", bufs=4) as opool,
            tc.tile_pool(name="psumA", bufs=3, space="PSUM") as psumA,
            tc.tile_pool(name="psumB", bufs=1, space="PSUM") as psumB,
        ):
            # ---------------- input DMAs, priority order ----------------
            pk = small.tile([D, PKW], F32)
            nc.sync.dma_start(pk, pk_d[:, :])
            xbkT_sb = small.tile([D, RB], F32)
            nc.sync.dma_start(xbkT_sb, xbkT_d[:, :])
            x_flat = persist.tile([128, NT * D], F32)
            x_sb = x_flat.rearrange("p (t d) -> p t d", t=NT)
            xchunks = [(0, 4), (4, 4), (8, 8), (16, 16), (32, 16), (48, 16)]
            for tw, nw in xchunks:
                nc.sync.dma_start(
                    x_flat[:, tw * D : (tw + nw) * D],
                    xp_d[:, tw * D : (tw + nw) * D],
                )

            W_sb = pk[0:D, 0:D]
            b_col = pk[0:D, D : D + 1]
            a_sb = pk[0:D, D + 1 : D + 3]
            a1col = pk[0:D, D + 1 : D + 2]
            a2col = pk[0:D, D + 2 : D + 3]

            # ---------------- on-device constants ----------------
            ones64 = small.tile([D, 128], F32)
            nc.vector.memset(ones64, 1.0)
            ones_bf = small.tile([1, 128], BF16)
            nc.vector.memset(ones_bf, 1.0)
            ident = small.tile([128, 128], F32)
            make_identity(nc, ident)  # gpsimd memset + affine_select

            # bf16 x_ext (x | ones | garbage), derived on device. Columns
            # 65..127 stay uninitialized: they only feed PSUM partitions
            # 65..127 which are never read.
            xbf_flat = persist.tile([128, NT * 128], BF16)
            x_bf = xbf_flat.rearrange("p (t d) -> p t d", t=NT)
            nc.vector.memset(x_bf[:, :, D : D + 1], 1.0)  # ones column
            # ACT warmup: trigger the activation table load at engine boot
            warm = small.tile([1, 1], F32)
            nc.vector.memset(warm, 0.0)
            warm2 = small.tile([1, 1], F32)
            nc.scalar.activation(out=warm2, in_=warm, func=EXP)

            # ---------------- v1 broadcast + c12, one ones-matmul ---------
            # vb_rhs = [ a1*W | b*(a1+a2) ]  (per-row scaling on DVE)
            # vb_ps[p, 0:64] = sum_k a1[k] W[k, d] = v1[d]  (all partitions)
            # vb_ps[p, 64]   = sum_k b[k](a1[k]+a2[k]) = c1+c2
            vb_rhs = small.tile([D, D + 1], F32)
            nc.vector.tensor_scalar(
                out=vb_rhs[:, 0:D], in0=W_sb, scalar1=a1col,
                scalar2=None, op0=MUL,
            )
            a12 = small.tile([D, 1], F32)
            nc.vector.tensor_add(a12, a1col, a2col)
            nc.vector.tensor_mul(vb_rhs[:, D : D + 1], b_col, a12)
            vb_ps = psumA.tile([128, D + 1], F32, tag="ps", name="vb_ps")
            nc.tensor.matmul(vb_ps, lhsT=ones64, rhs=vb_rhs, start=True, stop=True)
            vb_sb = small.tile([128, D + 1], F32)
            nc.scalar.copy(out=vb_sb, in_=vb_ps)
            v1b = vb_sb[:, 0:D]
            c12 = vb_sb[:, D : D + 1]
            c12s = small.tile([128, 1], F32)
            nc.vector.tensor_scalar(
                out=c12s, in0=c12, scalar1=0.01, scalar2=None, op0=MUL
            )


            # v2 column for the p2 projection
            v_ps = psumA.tile([D, 2], F32, tag="ps", name="v_ps")
            nc.tensor.matmul(v_ps, lhsT=W_sb, rhs=a_sb, start=True, stop=True)
            v_sb = small.tile([D, 2], F32)
            nc.scalar.copy(out=v_sb, in_=v_ps)

            # ---------------- p2 for this block -> G2b ----------------
            # p2row = v2.T @ xblk.T ; G2b[j,i] = exp(-0.99 * p2[i]) bcast
            G2b = persist.tile([128, RB], BF16)
            QW = 256
            for q in range(4):
                p2r_ps = psumA.tile([1, QW], F32, tag="ps", name="p2r_ps")
                nc.tensor.matmul(
                    p2r_ps,
                    lhsT=v_sb[:, 1:2],
                    rhs=xbkT_sb[:, q * QW : (q + 1) * QW],
                    start=True,
                    stop=True,
                )
                g2row = small.tile([1, QW], BF16, tag="g2row", name="g2row")
                nc.scalar.activation(out=g2row, in_=p2r_ps, func=EXP, scale=-0.99)
                gb_ps = psumA.tile([128, QW], F32, tag="ps", name="gb_ps")
                nc.tensor.matmul(
                    gb_ps, lhsT=ones_bf, rhs=g2row, start=True, stop=True
                )
                nc.vector.tensor_copy(G2b[:, q * QW : (q + 1) * QW], gb_ps)

            # ---------------- s1 products on GPSIMD (runs ahead) ----------
            # tmp[p, t, d] = x[p, t, d] * v1[d]
            schunks = [(0, 4), (4, 4)] + [(8 * c, 8) for c in range(1, 8)]

            def s1_products(ci):
                tw, nw = schunks[ci]
                v1b_b = bass.AP(
                    tensor=vb_sb.tensor,
                    offset=vb_sb.offset,
                    ap=[vb_sb.ap[0], [0, nw], [1, D]],
                )
                tmp = work.tile([128, nw, D], F32, tag="tmp", name="tmp")
                nc.gpsimd.tensor_mul(tmp, x_sb[:, tw : tw + nw, :], v1b_b)
                return tmp

            tmps = [s1_products(0), s1_products(1), s1_products(2)]

            # ---------------- ACT: x casts, chunk order -------------------
            s1c = small.tile([128, NT], F32)
            E1c = small.tile([128, NT], F32)
            F1c = small.tile([128, NT], F32)
            for tw, nw in schunks:
                # cast x chunk -> bf16 x_ext columns 0:64
                nc.scalar.copy(
                    out=x_bf[:, tw : tw + nw, 0:D],
                    in_=x_sb[:, tw : tw + nw, :],
                )

            # -------- main stream --------------------------------------
            # s1 reduction runs on ACT (Identity + accum_out, one per
            # j-tile) so the DVE queue holds ONLY the 64 e-tiles.
            SF = D + 1
            acc0 = psumB.tile([SF, 512], F32, tag="acc0", name="acc0")
            acc1 = psumB.tile([SF, 512], F32, tag="acc1", name="acc1")
            accs = [acc0, acc1]
            for ci, (tw, nw) in enumerate(schunks):
                nc.vector.tensor_reduce(
                    out=s1c[:, tw : tw + nw],
                    in_=tmps[ci],
                    axis=AX_X,
                    op=ADD,
                )
                nc.scalar.activation(
                    out=E1c[:, tw : tw + nw],
                    in_=s1c[:, tw : tw + nw],
                    func=EXP,
                    bias=c12,
                    scale=1.0,
                )
                nc.scalar.activation(
                    out=F1c[:, tw : tw + nw],
                    in_=s1c[:, tw : tw + nw],
                    func=EXP,
                    bias=c12s,
                    scale=0.01,
                )
                if ci + 3 < len(schunks):
                    tmps.append(s1_products(ci + 3))
                for jt in range(tw, tw + nw):
                    if ci == 0:
                        # first chunk: split per G2b half so the stream can
                        # start before the last G2b quarter lands
                        for h in range(2):
                            e_h = epool.tile(
                                [128, 512], BF16, tag="eh", name="e_h"
                            )
                            nc.vector.tensor_scalar(
                                out=e_h,
                                in0=G2b[:, h * 512 : (h + 1) * 512],
                                scalar1=F1c[:, jt : jt + 1],
                                scalar2=E1c[:, jt : jt + 1],
                                op0=MUL,
                                op1=MAX,
                            )
                            nc.tensor.matmul(
                                accs[h],
                                lhsT=x_bf[:, jt, 0:SF],
                                rhs=e_h,
                                start=(jt == 0),
                                stop=(jt == NT - 1),
                            )
                    else:
                        e_t = epool.tile([128, RB], BF16, tag="e", name="e_t")
                        # e[j,i] = max(G2b[j,i] * F1[j], E1[j])
                        nc.vector.tensor_scalar(
                            out=e_t,
                            in0=G2b,
                            scalar1=F1c[:, jt : jt + 1],
                            scalar2=E1c[:, jt : jt + 1],
                            op0=MUL,
                            op1=MAX,
                        )
                        for h in range(2):
                            nc.tensor.matmul(
                                accs[h],
                                lhsT=x_bf[:, jt, 0:SF],
                                rhs=e_t[:, h * 512 : (h + 1) * 512],
                                start=(jt == 0),
                                stop=(jt == NT - 1),
                            )

            # ---------------- epilogue: normalize + store ----------------
            outT = small.tile([D + 1, RB], F32)
            for h in range(2):
                for qq in range(2):
                    nc.scalar.copy(
                        out=outT[
                            :, h * 512 + qq * 256 : h * 512 + (qq + 1) * 256
                        ],
                        in_=accs[h][:, qq * 256 : (qq + 1) * 256],
                    )
            out_flat = small.tile([128, BT * D], F32)
            out_sb = out_flat.rearrange("p (t d) -> p t d", t=BT)
            for t in range(BT):
                tp2 = psumA.tile([128, D + 1], F32, tag="ps", name="tp2")
                nc.tensor.transpose(
                    tp2, outT[:, t * 128 : (t + 1) * 128], ident[: D + 1, : D + 1]
                )
                rcol = opool.tile([128, 1], F32, tag="rcol", name="rcol")
                nc.vector.reciprocal(rcol, tp2[:, D : D + 1])
                if t % 2 == 0:
                    nc.vector.tensor_scalar(
                        out=out_sb[:, t, :],
                        in0=tp2[:, 0:D],
                        scalar1=rcol,
                        scalar2=None,
                        op0=MUL,
                    )
                else:
                    nc.scalar.mul(out_sb[:, t, :], tp2[:, 0:D], rcol)
                if t == 3:
                    nc.sync.dma_start(out_d[:, : 4 * D], out_flat[:, : 4 * D])
                elif t == 6:
                    nc.sync.dma_start(
                        out_d[:, 4 * D : 7 * D], out_flat[:, 4 * D : 7 * D]
                    )
            nc.sync.dma_start(out_d[:, 7 * D :], out_flat[:, 7 * D :])

    nc.finalize()
    return nc


def _execute(inputs: dict, trace: bool = False):
    x = np.ascontiguousarray(np.asarray(inputs["x"], dtype=np.float32))
    W = np.ascontiguousarray(np.asarray(inputs["W"], dtype=np.float32))
    b = np.ascontiguousarray(
        np.asarray(inputs["b"], dtype=np.float32).reshape(D, 1)
    )
    a = np.ascontiguousarray(
        np.asarray(inputs["a"], dtype=np.float32).reshape(2 * D, 1)
    )
    assert x.shape == (N, D) and W.shape == (D, D)

    # partition-major permutation: (t*128+p, d) -> (p, t*D+d)
    xp = np.ascontiguousarray(
        x.reshape(NT, 128, D).transpose(1, 0, 2).reshape(128, NT * D)
    )
    nc = build_bass()
    pack0 = np.zeros((D, PKW), np.float32)
    pack0[:, 0:D] = W
    pack0[:, D] = b[:, 0]
    pack0[:, D + 1] = a[:D, 0]
    pack0[:, D + 2] = a[D:, 0]
    in_maps = []
    for c in range(NCORES):
        xbkT = np.ascontiguousarray(x[c * RB : (c + 1) * RB].T)
        in_maps.append({"xp": xp, "pack": pack0, "xbkT": xbkT})
    res = run_bass_kernel_spmd(
        nc, in_maps, core_ids=list(range(NCORES)), trace=trace
    )
    # un-permute each core's output: (p, t*D+d) -> (t*128+p, d)
    outs = []
    for r in res.results:
        o = r["out"].reshape(128, BT, D).transpose(1, 0, 2).reshape(RB, D)
        outs.append(o)
    out = np.ascontiguousarray(np.concatenate(outs, axis=0))
    return out, res


def kernel(x, W, b, a):
    out, _ = _execute({"x": x, "W": W, "b": b, "a": a})
    return out
